# revision 4
# baseline (speedup 1.0000x reference)
"""LocallyConnected2D (B=16, 32x32, CIN=COUT=64, 3x3, pad=1) on 8 TRN2 NeuronCores.

Shard the 32 output rows across 8 cores (4 rows each). Weights ride to the
device as fp8 e3m4 (scaled x32; exact-input rel err 1.45e-2 < 2e-2), x as
fp16, fp32 PSUM accumulate — the matmul moving operand is the weights, so
fp8 halves the dominant HBM stream (9.24 -> 4.62 MB/core/rep) while the
fp16 stationary x keeps its full precision (bass allows mixed operand
dtypes).

Column-stationary formulation: input column c feeds the dj=2,1,0 taps of
output pixels c-1, c, c+1, so ONE matmul with stationary lhsT = x[:, c]
streams the weights of up to 3 adjacent pixels (N<=192). Row taps are
packed on the contraction axis: di=(0,1) as K=128 pairs (x rows r, r+1
stacked on partition halves = panel V(r)), di=2 as K=64 solo matmuls whose
weights ride partition half 64*(c%2) so the weight DMA stays 128-partition.
Taps that multiply the zero column padding are skipped entirely (their
weights are never sent). ~38 matmuls per (row, structure) instead of 160
tiny ones; 4 psum column strips of 8 consecutive pixels run concurrently
via tile_position.

out[b,i,j,o] = sum_{c,k} x_pad[b, i+di, j+dj, cin] * W[o,cin,i,j,3di+dj].

Host layouts (per core C, local row r, i = 4C+r, strip s = j//8, f = j%8):
  w_pairs [4, 128, 6016] f8: [64*di+cin, flat] = 32*W[o,cin,i,p,dj+3di], di=0,1
  w_solo  [4, 128, 3008] f8: [64*(c%2)+cin, flat] = 32*W[o,cin,i,p,6+dj]
  (flat, in matmul schedule order: c=0..31, strip pieces, pixels asc, o asc)
  xt      [384, 512] f16:    [rin*64+cin, j*16+b] = x_pad[b, 4C+rin, j, cin]
  out     [4, 16, 2048] fp16: [s, b, r*512 + f*64 + o] = 32*out[b, i, 8s+f, o]

x panels V(k) [128, 512], k=0..4: partitions = x rows (k, k+1), col c at
c*16 (no padding columns needed); S5 [64, 512] = row 5. Pair lhsT =
V(r)[:, c*16:+16]; solo lhsT = row r+2: V(r+2)[0:64] / S5 (c even),
V(r+1)[64:128] (c odd).

PSUM: one [128, 512] bank per r, DVE-zeroed first (all matmuls
start=False accumulate); one full-partition DVE cast f32->fp16 per r into
stage (unused partition lanes carry zeros); streamed [16, 2KB] output DMAs.

n_reps unrolled reps sit inside an optional tc.For_i(0, loop_iters) device
loop so the bench can run thousands of reps per dispatch (the axon tunnel
has ~100ms dispatch jitter; on-device looping makes the timing signal
dominate it).
"""

import numpy as np
import ml_dtypes

B, IH, IW, CIN = 16, 32, 32, 64
COUT, OH, OW = 64, 32, 32
NCORES, RPC = 8, 4
W_SCALE = 32.0

_NC = None


def _schedule():
    """Matmul schedule: list of q-groups, each a list of (c, s, p_lo, npix)
    pieces. Columns are interleaved across the 4 strips (c = 8s+q for
    q=0..7) so consecutive matmuls hit different PE column groups and
    overlapping accumulate regions are several instructions apart. Shared
    by the kernel builder and the host weight packer so the flat weight
    layout matches consumption order exactly."""
    groups = []
    for q in range(8):
        grp = []
        for st in range(4):
            c = 8 * st + q
            pixels = [p for p in (c - 1, c, c + 1) if 0 <= p < 32]
            run = []
            for p in pixels:
                if run and (p // 8 != run[0] // 8):
                    grp.append((c, run[0] // 8, run[0], len(run)))
                    run = []
                run.append(p)
            if run:
                grp.append((c, run[0] // 8, run[0], len(run)))
        groups.append(grp)
    return groups


def _build_nc(n_reps=1, loop_iters=1):
    import concourse.bacc as bacc
    import concourse.mybir as mybir
    import concourse.tile as tile

    f16 = mybir.dt.float16
    f32 = mybir.dt.float32
    f8 = mybir.dt.float8e3
    groups = _schedule()
    ntap = sum(npix for g in groups for _, _, _, npix in g)  # 94
    nc = bacc.Bacc("TRN2", target_bir_lowering=False, debug=False)
    wp = nc.dram_tensor("w_pairs", [RPC, 128, ntap * 64], f8, kind="ExternalInput")
    wso = nc.dram_tensor("w_solo", [RPC, 128, ntap * 32], f8, kind="ExternalInput")
    xt = nc.dram_tensor("xt", [384, 512], f16, kind="ExternalInput")
    out = nc.dram_tensor("out", [4, 16, RPC * 512], f16, kind="ExternalOutput")
    wp_ap, wso_ap, xt_ap, out_ap = wp.ap(), wso.ap(), xt.ap(), out.ap()

    with tile.TileContext(nc) as tc:
        with (
            tc.tile_pool(name="wp", bufs=3) as wp_pool,
            tc.tile_pool(name="wso", bufs=3) as wso_pool,
            tc.tile_pool(name="vx", bufs=2) as vx_pool,
            tc.tile_pool(name="stage", bufs=2) as stage_pool,
            tc.tile_pool(name="psum", bufs=8, space="PSUM") as psum_pool,
        ):

            def body():
                # x panels straight from HBM: V(k) covers x_pad rows (k, k+1)
                # so odd panels are just offset slices of the same xt buffer.
                # 6 DMAs instead of 3 loads + 5 SBUF->SBUF copies.
                vs = []
                for k in range(5):
                    v = vx_pool.tile([128, 512], f16, tag=f"v{k}")
                    vs.append(v)
                s5 = vx_pool.tile([64, 512], f16, tag="s5")
                for k in (0, 2, 4):
                    nc.sync.dma_start(vs[k][:], xt_ap[64 * k : 64 * k + 128])
                for k in (1, 3):
                    nc.scalar.dma_start(vs[k][:], xt_ap[64 * k : 64 * k + 128])
                nc.scalar.dma_start(s5[:], xt_ap[320:384])

                stage = stage_pool.tile([128, 2048], f16, tag="stage")
                for r in range(RPC):
                    wp_t = wp_pool.tile([128, ntap * 64], f8, tag="wp")
                    wso_t = wso_pool.tile([128, ntap * 32], f8, tag="wso")
                    weng = nc.sync if r < 2 else nc.scalar
                    weng.dma_start(wp_t[:], wp_ap[r][:])
                    nc.gpsimd.dma_start(wso_t[:], wso_ap[r][:])

                    ps = psum_pool.tile([128, 512], f32, tag="ps")
                    nc.vector.memset(ps[:], 0.0)
                    off = 0
                    soff = [0, 0]
                    nmm = sum(1 for g in groups for _ in g)
                    mm_i = 0
                    for grp in groups:
                        poffs, soffs = [], []
                        for c, s, p_lo, npix in grp:
                            n = npix * 64
                            poffs.append(off)
                            soffs.append(soff[c % 2])
                            off += n
                            soff[c % 2] += n
                        # pair pass: di=(0,1), K=128
                        for (c, s, p_lo, npix), po in zip(grp, poffs):
                            n = npix * 64
                            pslice = ps[
                                32 * s : 32 * s + 16,
                                (p_lo % 8) * 64 : (p_lo % 8) * 64 + n,
                            ]
                            nc.tensor.matmul(
                                pslice,
                                vs[r][:, c * 16 : (c + 1) * 16],
                                wp_t[:, po : po + n],
                                start=False,
                                stop=False,
                                tile_position=(0, 32 * s),
                                skip_group_check=True,
                            )
                        # solo pass: di=2, K=64, weights on half 64*(c%2)
                        for (c, s, p_lo, npix), so in zip(grp, soffs):
                            n = npix * 64
                            mm_i += 1
                            pslice = ps[
                                32 * s : 32 * s + 16,
                                (p_lo % 8) * 64 : (p_lo % 8) * 64 + n,
                            ]
                            if c % 2 == 0:
                                xsrc = (s5 if r == 3 else vs[r + 2])[
                                    0:64, c * 16 : (c + 1) * 16
                                ]
                                wsrc = wso_t[0:64, so : so + n]
                                tp = (0, 32 * s)
                            else:
                                xsrc = vs[r + 1][64:128, c * 16 : (c + 1) * 16]
                                wsrc = wso_t[64:128, so : so + n]
                                tp = (64, 32 * s)
                            nc.tensor.matmul(
                                pslice,
                                xsrc,
                                wsrc,
                                start=False,
                                stop=(mm_i == nmm),
                                tile_position=tp,
                                skip_group_check=True,
                            )
                    # Full-partition cast: unused lanes were zeroed by the
                    # memset, so one [128, 512] copy replaces 4 [16, 512]s.
                    nc.vector.tensor_copy(
                        stage[:, r * 512 : (r + 1) * 512], ps[:, :]
                    )
                    if r % 2 == 1:
                        for s in range(4):
                            oeng = nc.sync if s < 2 else nc.scalar
                            oeng.dma_start(
                                out_ap[s][:, (r - 1) * 512 : (r + 1) * 512],
                                stage[
                                    32 * s : 32 * s + 16,
                                    (r - 1) * 512 : (r + 1) * 512,
                                ],
                            )

            if loop_iters > 1:
                with tc.For_i(0, loop_iters):
                    for _ in range(n_reps):
                        body()
            else:
                for _ in range(n_reps):
                    body()
    nc.compile()
    return nc


def _repack_inputs(x, weight):
    x = np.asarray(x, dtype=np.float32)
    weight = np.asarray(weight, dtype=np.float32)
    sched = [piece for grp in _schedule() for piece in grp]
    ntap = sum(npix for _, _, _, npix in sched)

    # wt[i, cin, o, j, k]
    wt = np.ascontiguousarray(weight.transpose(2, 1, 0, 3, 4)) * W_SCALE
    wpair = np.zeros((OH, 128, ntap * 64), dtype=np.float32)
    wsolo = np.zeros((OH, 128, ntap * 32), dtype=np.float32)
    off = 0
    soff = [0, 0]
    for c, s, p_lo, npix in sched:
        for e, p in enumerate(range(p_lo, p_lo + npix)):
            dj = c - p + 1
            pb = slice(off + 64 * e, off + 64 * (e + 1))
            sb = slice(soff[c % 2] + 64 * e, soff[c % 2] + 64 * (e + 1))
            wpair[:, 0:64, pb] = wt[:, :, :, p, dj]  # di=0
            wpair[:, 64:128, pb] = wt[:, :, :, p, 3 + dj]  # di=1
            half = 64 * (c % 2)
            wsolo[:, half : half + 64, sb] = wt[:, :, :, p, 6 + dj]  # di=2
        off += 64 * npix
        soff[c % 2] += 64 * npix
    wpair = wpair.astype(ml_dtypes.float8_e3m4)
    wsolo = wsolo.astype(ml_dtypes.float8_e3m4)

    xpad = np.zeros((IH + 2, CIN, IW, B), dtype=np.float16)
    xpad[1:33] = x.transpose(1, 3, 2, 0)  # [ih, c, j, b]

    in_maps = []
    for c in range(NCORES):
        in_maps.append(
            {
                "w_pairs": np.ascontiguousarray(wpair[c * RPC : (c + 1) * RPC]),
                "w_solo": np.ascontiguousarray(wsolo[c * RPC : (c + 1) * RPC]),
                "xt": np.ascontiguousarray(
                    xpad[c * RPC : c * RPC + RPC + 2].reshape(384, 512)
                ),
            }
        )
    return in_maps


def _get_nc():
    global _NC
    if _NC is None:
        _NC = _build_nc()
    return _NC


def run_spmd(in_maps, **kwargs):
    from concourse.bass_utils import run_bass_kernel_spmd

    return run_bass_kernel_spmd(
        _get_nc(), in_maps, core_ids=list(range(NCORES)), **kwargs
    )


def kernel(x, weight, bias, _results=None):
    if _results is None:
        _results = run_spmd(_repack_inputs(x, weight)).results
    arr = np.stack([r["out"] for r in _results]).astype(np.float32) / W_SCALE
    arr = arr.reshape(NCORES, 4, 16, RPC, 8, 64)
    # arr: [core, s, b, r, f, o] -> out[b, 4*core+r, 8s+f, o]
    out = arr.transpose(2, 0, 3, 1, 4, 5).reshape(B, OH, OW, COUT)
    return out + np.asarray(bias, dtype=np.float32)[None]


# revision 6
# speedup vs baseline: 1.4533x; 1.4533x over previous
"""LocallyConnected2D (B=16, 32x32, CIN=COUT=64, 3x3, pad=1) on 8 TRN2 NeuronCores.

Shard the 32 output rows across 8 cores (4 rows each). Weights ride to the
device as fp8 e3m4 (scaled x32; exact-input rel err 1.45e-2 < 2e-2), x as
fp16, fp32 PSUM accumulate — the matmul moving operand is the weights, so
fp8 halves the dominant HBM stream (9.24 -> 4.62 MB/core/rep) while the
fp16 stationary x keeps full precision (bass allows mixed operand dtypes).

out[b,i,j,o] = sum_{c,k} x_pad[b, i+di, j+dj, cin] * W[o,cin,i,j,3di+dj].

Column-stationary formulation: input column c feeds the dj=2,1,0 taps of
output pixels c-1, c, c+1, so ONE matmul with stationary lhsT = x[:, c]
streams the weights of up to 3 adjacent pixels (N<=192).

Row taps on the contraction axis, all K=128:
  * pairs: di=(0,1) for row r -> stationary V(r) = x rows (r, r+1) stacked
    on partition halves, M=16 (batch).
  * solos: di=2 for rows (2p, 2p+1) merged -> block-diagonal stationary
    B(p) = diag(x row 2p+2, x row 2p+3), M=32 (16 batch x 2 rows); the
    weight stream stays fully dense. Matmul outputs can only land at psum
    partition offsets = 0 mod 32, so merged-solo results (row 2p+1 at
    offset 16 mod 32) go to separate psum banks, are staged separately,
    and the host adds the pair and solo output streams.

Per rep per core: 152 pair + 76 solo matmuls, 36.1k PE streaming cycles
(the K=128 floor for 4.62M weight elements).

Host layouts (per core C, local row r, i = 4C+r, strip s = j//8, f = j%8;
flat = matmul schedule order: q-groups, strip pieces, pixels asc, o asc):
  w_pairs [4, 128, 6016] f8: [64*di+cin, flat] = 32*W[o,cin,i,p,dj+3di]
  w_solo  [2, 128, 6016] f8: [64*rr+cin, flat] = 32*W[o,cin,4C+2p+rr,p,6+dj]
  xt      [384, 512] f16:    [l*64+cin, j*16+b] = x_pad[b, 4C+l, j, c]
  xbd     [2, 128, 1024] f16: [64*rr+cin, c*32+16*rr+b] = x_pad[b, 4C+2p+2+rr, c, cin]
  out     [4, 16, 2048] f16:  [s, b, r*512+f*64+o]    = 32*pair_part
  out2    [2, 128, 512] f16:  [p, 32*s+16*rr+b, f*64+o] = 32*solo_part

PSUM: per rep 4 pair banks + 2 solo banks [128, 512] f32, DVE-zeroed (all
matmuls accumulate with start=False); full-partition DVE casts f32->f16
into stage tiles; [16|128, 1-4KB] output DMAs split across SP/ACT rings;
weight DMAs split SP/ACT (pairs) + gpsimd SWDGE (solos, xbd).

n_reps unrolled reps sit inside an optional tc.For_i(0, loop_iters) device
loop so the bench can run thousands of reps per dispatch (the axon tunnel
has ~100ms dispatch jitter; on-device looping makes the timing signal
dominate it).
"""

import numpy as np
import ml_dtypes

B, IH, IW, CIN = 16, 32, 32, 64
COUT, OH, OW = 64, 32, 32
NCORES, RPC = 8, 4
W_SCALE = 32.0

_NC = None


def _schedule():
    """Matmul schedule: list of q-groups, each a list of (c, s, p_lo, npix)
    pieces. Columns are interleaved across the 4 strips (c = 8s+q for
    q=0..7) so consecutive matmuls hit different PE column groups and
    overlapping accumulate regions are several instructions apart. Shared
    by the kernel builder and the host weight packer so the flat weight
    layout matches consumption order exactly."""
    groups = []
    for q in range(8):
        grp = []
        for st in range(4):
            c = 8 * st + q
            pixels = [p for p in (c - 1, c, c + 1) if 0 <= p < 32]
            run = []
            for p in pixels:
                if run and (p // 8 != run[0] // 8):
                    grp.append((c, run[0] // 8, run[0], len(run)))
                    run = []
                run.append(p)
            if run:
                grp.append((c, run[0] // 8, run[0], len(run)))
        groups.append(grp)
    return groups


def _build_nc(n_reps=1, loop_iters=1):
    import concourse.bacc as bacc
    import concourse.mybir as mybir
    import concourse.tile as tile

    f16 = mybir.dt.float16
    f32 = mybir.dt.float32
    f8 = mybir.dt.float8e3
    groups = _schedule()
    pieces = [piece for g in groups for piece in g]
    ntap = sum(npix for _, _, _, npix in pieces)  # 94
    nmm = len(pieces)
    nc = bacc.Bacc("TRN2", target_bir_lowering=False, debug=False)
    wp = nc.dram_tensor("w_pairs", [RPC, 128, ntap * 64], f8, kind="ExternalInput")
    wso = nc.dram_tensor("w_solo", [2, 128, ntap * 64], f8, kind="ExternalInput")
    xt = nc.dram_tensor("xt", [384, 512], f16, kind="ExternalInput")
    xbd = nc.dram_tensor("xbd", [2, 128, 1024], f16, kind="ExternalInput")
    out = nc.dram_tensor("out", [4, 16, RPC * 512], f16, kind="ExternalOutput")
    out2 = nc.dram_tensor("out2", [2, 128, 512], f16, kind="ExternalOutput")
    wp_ap, wso_ap, xt_ap, xbd_ap = wp.ap(), wso.ap(), xt.ap(), xbd.ap()
    out_ap, out2_ap = out.ap(), out2.ap()

    # flat column offsets per piece, in schedule order
    offs = np.cumsum([0] + [npix * 64 for _, _, _, npix in pieces])

    with tile.TileContext(nc) as tc:
        with (
            tc.tile_pool(name="wp", bufs=3) as wp_pool,
            tc.tile_pool(name="wso", bufs=2) as wso_pool,
            tc.tile_pool(name="vx", bufs=2) as vx_pool,
            tc.tile_pool(name="stage", bufs=2) as stage_pool,
            tc.tile_pool(name="psum_p", bufs=6, space="PSUM") as psum_p_pool,
            tc.tile_pool(name="psum_s", bufs=2, space="PSUM") as psum_s_pool,
        ):

            def emit_pairs(r, vs, stage):
                wp_t = wp_pool.tile([128, ntap * 64], f8, tag="wp")
                weng = nc.sync if r < 2 else nc.scalar
                weng.dma_start(wp_t[:], wp_ap[r][:])
                ps = psum_p_pool.tile([128, 512], f32, tag="psp")
                nc.vector.memset(ps[:], 0.0)
                for mi, (c, s, p_lo, npix) in enumerate(pieces):
                    n = npix * 64
                    po = int(offs[mi])
                    pslice = ps[
                        32 * s : 32 * s + 16,
                        (p_lo % 8) * 64 : (p_lo % 8) * 64 + n,
                    ]
                    nc.tensor.matmul(
                        pslice,
                        vs[r][:, c * 16 : (c + 1) * 16],
                        wp_t[:, po : po + n],
                        start=False,
                        stop=(mi == nmm - 1),
                        tile_position=(0, 32 * s),
                        skip_group_check=True,
                    )
                # Full-partition cast: unused lanes carry memset zeros, so
                # one [128, 512] copy replaces 4 [16, 512]s.
                nc.vector.tensor_copy(stage[:, r * 512 : (r + 1) * 512], ps[:, :])
                if r % 2 == 1:
                    for s in range(4):
                        oeng = nc.sync if s < 2 else nc.scalar
                        oeng.dma_start(
                            out_ap[s][:, (r - 1) * 512 : (r + 1) * 512],
                            stage[
                                32 * s : 32 * s + 16,
                                (r - 1) * 512 : (r + 1) * 512,
                            ],
                        )

            def emit_solo(p, bd, stage2):
                wso_t = wso_pool.tile([128, ntap * 64], f8, tag="wso")
                nc.gpsimd.dma_start(wso_t[:], wso_ap[p][:])
                ps = psum_s_pool.tile([128, 512], f32, tag="pss")
                nc.vector.memset(ps[:], 0.0)
                for mi, (c, s, p_lo, npix) in enumerate(pieces):
                    n = npix * 64
                    po = int(offs[mi])
                    pslice = ps[
                        32 * s : 32 * s + 32,
                        (p_lo % 8) * 64 : (p_lo % 8) * 64 + n,
                    ]
                    nc.tensor.matmul(
                        pslice,
                        bd[:, c * 32 : (c + 1) * 32],
                        wso_t[:, po : po + n],
                        start=False,
                        stop=(mi == nmm - 1),
                        tile_position=(0, 32 * s),
                        skip_group_check=True,
                    )
                nc.vector.tensor_copy(stage2[:, p * 512 : (p + 1) * 512], ps[:, :])
                nc.scalar.dma_start(
                    out2_ap[p][:], stage2[:, p * 512 : (p + 1) * 512]
                )

            def body():
                # x panels straight from HBM: V(k) covers x_pad rows (k, k+1)
                # at partition offset 64k of the same xt buffer.
                vs = []
                for k in range(5):
                    v = vx_pool.tile([128, 512], f16, tag=f"v{k}")
                    vs.append(v)
                bds = []
                for p in range(2):
                    bd = vx_pool.tile([128, 1024], f16, tag=f"bd{p}")
                    bds.append(bd)
                    nc.gpsimd.dma_start(bd[:], xbd_ap[p][:])
                for k in (0, 2, 4):
                    nc.sync.dma_start(vs[k][:], xt_ap[64 * k : 64 * k + 128])
                for k in (1, 3):
                    nc.scalar.dma_start(vs[k][:], xt_ap[64 * k : 64 * k + 128])

                stage = stage_pool.tile([128, 2048], f16, tag="stage")
                stage2 = stage_pool.tile([128, 1024], f16, tag="stage2")
                emit_pairs(0, vs, stage)
                emit_pairs(1, vs, stage)
                emit_solo(0, bds[0], stage2)
                emit_pairs(2, vs, stage)
                emit_pairs(3, vs, stage)
                emit_solo(1, bds[1], stage2)

            if loop_iters > 1:
                with tc.For_i(0, loop_iters):
                    for _ in range(n_reps):
                        body()
            else:
                for _ in range(n_reps):
                    body()
    nc.compile()
    return nc


def _repack_inputs(x, weight):
    x = np.asarray(x, dtype=np.float32)
    weight = np.asarray(weight, dtype=np.float32)
    pieces = [piece for grp in _schedule() for piece in grp]
    ntap = sum(npix for _, _, _, npix in pieces)

    # wt[i, cin, o, j, k]
    wt = np.ascontiguousarray(weight.transpose(2, 1, 0, 3, 4)) * W_SCALE
    wpair = np.zeros((OH, 128, ntap * 64), dtype=np.float32)
    wsolo = np.zeros((OH // 2, 128, ntap * 64), dtype=np.float32)
    off = 0
    for c, s, p_lo, npix in pieces:
        for e, p in enumerate(range(p_lo, p_lo + npix)):
            dj = c - p + 1
            pb = slice(off + 64 * e, off + 64 * (e + 1))
            wpair[:, 0:64, pb] = wt[:, :, :, p, dj]  # di=0
            wpair[:, 64:128, pb] = wt[:, :, :, p, 3 + dj]  # di=1
            # solo di=2, rows (2p, 2p+1) on partition halves
            wsolo[:, 0:64, pb] = wt[0::2, :, :, p, 6 + dj]
            wsolo[:, 64:128, pb] = wt[1::2, :, :, p, 6 + dj]
        off += 64 * npix
    wpair = wpair.astype(ml_dtypes.float8_e3m4)
    wsolo = wsolo.astype(ml_dtypes.float8_e3m4)

    xpad = np.zeros((IH + 2, CIN, IW, B), dtype=np.float16)
    xpad[1:33] = x.transpose(1, 3, 2, 0)  # [ih, c, j, b]

    # block-diagonal solo stationaries: B(p) = diag(row 2p+2, row 2p+3)
    xbd = np.zeros((NCORES, 2, 2, CIN, IW, 2, 16), dtype=np.float16)
    for C in range(NCORES):
        for p in range(2):
            for rr in range(2):
                xbd[C, p, rr, :, :, rr, :] = xpad[4 * C + 2 * p + 2 + rr]
    xbd = xbd.reshape(NCORES, 2, 128, 1024)

    in_maps = []
    for c in range(NCORES):
        in_maps.append(
            {
                "w_pairs": np.ascontiguousarray(wpair[c * RPC : (c + 1) * RPC]),
                "w_solo": np.ascontiguousarray(wsolo[2 * c : 2 * c + 2]),
                "xt": np.ascontiguousarray(
                    xpad[c * RPC : c * RPC + RPC + 2].reshape(384, 512)
                ),
                "xbd": xbd[c],
            }
        )
    return in_maps


def _get_nc():
    global _NC
    if _NC is None:
        _NC = _build_nc()
    return _NC


def run_spmd(in_maps, **kwargs):
    from concourse.bass_utils import run_bass_kernel_spmd

    return run_bass_kernel_spmd(
        _get_nc(), in_maps, core_ids=list(range(NCORES)), **kwargs
    )


def kernel(x, weight, bias, _results=None):
    if _results is None:
        _results = run_spmd(_repack_inputs(x, weight)).results
    arr = np.stack([r["out"] for r in _results]).astype(np.float32)
    arr = arr.reshape(NCORES, 4, 16, RPC, 8, 64)
    # arr: [core, s, b, r, f, o] -> out[b, 4*core+r, 8s+f, o]
    out = arr.transpose(2, 0, 3, 1, 4, 5).reshape(B, OH, OW, COUT)
    # solo stream: [core, p, 32s+16rr+b, f*64+o] -> out[b, 4*core+2p+rr, 8s+f, o]
    arr2 = np.stack([r["out2"] for r in _results]).astype(np.float32)
    arr2 = arr2.reshape(NCORES, 2, 4, 2, 16, 8, 64)
    out2 = arr2.transpose(4, 0, 1, 3, 2, 5, 6).reshape(B, OH, OW, COUT)
    return (out + out2) / W_SCALE + np.asarray(bias, dtype=np.float32)[None]
